# revision 37
# baseline (speedup 1.0000x reference)
"""Trainium2 Bass kernel for nn_BaseConchNc (GNN message passing), 8-core SPMD.

Architecture (per core, feature-major pipeline):
  - nodes sharded 8 ways; each core processes SH nodes (padded to PADSH, tiles of 256)
  - neighbor-mean rewritten as mean(table[neigh]) with table = x @ (Wn/16)
    (mean commutes with the linear projection; relu applied after)
  - tables (y1, y2 per metapath) are bf16 in DRAM, exchanged via per-mp
    AllGather (contributions 128-row aligned, CON rows per rank)
  - gathers: non-transpose DRAM-source dma_gather (single_packet=False, one
    call per tile+window); int16 index limit 32767 handled by two windows
    (A = tokens < WINB, B = rest) with per-node neighbor lists split A-first
    on host; per-tile rectangular slot counts KA[t]/KB[t] (host snake-sorts
    nodes by window counts to keep padding small); pad slots point at
    all-zero dummy tokens; k-major positions (blk*K+k)*128+j put node j's K
    neighbor rows contiguously on partition j
  - 16-neighbor sum: one DVE tensor_reduce over k (strided view [p,b,f,k]);
    result is node-major, so the nh half of each output row needs no transpose
  - schedule: P -> AG(y1-0),AG(y1-1) -> L1(0) -> AG(y2-0) -> L1(1) ->
    AG(y2-1) -> xh2-mp0 precompute (overlaps AG) -> L2(0) -> L2(1);
    h1 kept resident per mp (bf16 feature-major chunks)
  - all device I/O is packed into ONE ExternalInput blob and ONE bf16 output
    (per-argument dispatch overhead dominates small-kernel timing here)
"""
import sys
sys.path.insert(0, '/opt/trn_rl_repo')
import numpy as np
import ml_dtypes

import concourse.bacc as bacc
import concourse.mybir as mybir
import concourse.tile as tile
from concourse import library_config
from concourse.masks import make_identity
from concourse.bass_utils import run_bass_kernel_spmd

F32 = mybir.dt.float32
BF16 = mybir.dt.bfloat16
I16 = mybir.dt.int16


def default_cfg():
    return dict(
        N=50000, S=16, DIN=256, PREP=128, H=2, OUT=64, MP=2,
        NC=8, TILE=256, WINB=31232,
    )


def derive(cfg):
    c = dict(cfg)
    c["IN2"] = 2 * c["OUT"] * c["H"]          # 256
    c["F"] = c["OUT"] * c["H"]                # 128 (table width / compact width)
    assert c["N"] % c["NC"] == 0
    c["SH"] = c["N"] // c["NC"]
    c["NT"] = -(-c["SH"] // c["TILE"])
    c["PADSH"] = c["NT"] * c["TILE"]
    # per-rank, per-mp AG contribution rows; 128-aligned so each (rank, mp)
    # block is a whole number of 128-token stripes (rows SH..CON-1 are zero)
    c["CON"] = -(-(c["SH"] + 1) // 128) * 128
    c["NTOK"] = c["NC"] * c["CON"]
    c["STRIPES"] = c["NTOK"] // 128
    assert c["NTOK"] % 128 == 0 and c["CON"] % 128 == 0
    assert c["WINB"] % 128 == 0
    # zero tokens: position SH of rank 0 (window A) / of first rank past WINB (window B)
    c["ZA"] = c["SH"]
    rz = 0
    while rz * c["CON"] + c["SH"] < c["WINB"]:
        rz += 1
    c["ZB"] = rz * c["CON"] + c["SH"] - c["WINB"]
    assert c["ZA"] < c["WINB"] <= 32768
    assert 0 <= c["ZB"] <= 32767
    assert c["NTOK"] - c["WINB"] <= 32768
    return c


def blob_layout(c, KA, KB):
    """Section offsets (int16 units) inside the packed input blob."""
    MP, TILE = c["MP"], c["TILE"]
    secs = {}
    off = 0

    def add(name, rows, cols):
        nonlocal off
        off = -(-off // 256) * 256            # 512B-align each section
        secs[name] = (off, rows, cols)
        off += rows * cols

    add("featsT", c["DIN"], c["PADSH"])       # bf16
    add("prepW", c["DIN"], c["PREP"])         # bf16
    add("wx1", MP * c["PREP"], c["F"])        # bf16
    add("wn1", MP * c["PREP"], c["F"])        # bf16
    add("wx2", MP * c["IN2"], c["F"])         # bf16
    add("wn2", MP * c["IN2"], c["F"])         # bf16
    for mp in range(MP):
        add(f"idxa{mp}", 16, int(KA[mp].sum()) * TILE // 16)
        add(f"idxb{mp}", 16, int(KB[mp].sum()) * TILE // 16)
    total = -(-off // 256) * 256
    return secs, total


# ---------------------------------------------------------------- host prep

def host_prep(cfg, feats, prep_W, Wx1, Wn1, Wx2, Wn2, node_neigh):
    c = derive(cfg)
    N, S, MP, NC, SH, TILE, NT, PADSH = (c[k] for k in
        ("N", "S", "MP", "NC", "SH", "TILE", "NT", "PADSH"))
    CON, WINB, F, OUT = c["CON"], c["WINB"], c["F"], c["OUT"]

    nb = np.asarray(node_neigh, np.int64)            # [MP, N, S]
    own = np.arange(N) // SH

    # two sort passes: provisional counts -> perms -> exact counts -> re-sort.
    # WINB sits just below a rank boundary, so only a handful of top-sorted
    # positions of one core can flip window membership between passes.
    def make_tau(perm):
        tau = np.empty(N, np.int64)
        for r in range(NC):
            tau[r * SH + perm[r]] = r * CON + np.arange(SH)
        return tau

    def snake_key(k):
        return k[0] * (4 * S) + np.where(k[0] % 2 == 0, k[1], S - k[1])

    tau0 = own * CON + (np.arange(N) - own * SH)
    kA_p = (tau0[nb] < WINB).sum(-1)                 # [MP, N] provisional
    perm = np.stack([np.argsort(snake_key(kA_p[:, r * SH:(r + 1) * SH]),
                                kind="stable") for r in range(NC)])
    kA_p = (make_tau(perm)[nb] < WINB).sum(-1)
    perm = np.stack([np.argsort(snake_key(kA_p[:, r * SH:(r + 1) * SH]),
                                kind="stable") for r in range(NC)])
    tau = make_tau(perm)

    # final window membership / counts
    tau_nb = tau[nb]                                 # [MP, N, S]
    winA = tau_nb < WINB
    kA = winA.sum(-1)                                # [MP, N]
    kB = S - kA

    # neighbor tokens, window-A entries first (stable)
    order = np.argsort(~winA, axis=-1, kind="stable")
    ts = np.take_along_axis(tau_nb, order, axis=-1)  # [MP, N, S]

    # per-tile slot counts, max over cores (program is shared across cores)
    KA = np.zeros((MP, NT), np.int64)
    KB = np.zeros((MP, NT), np.int64)
    for r in range(NC):
        ka_p = np.zeros((MP, PADSH), np.int64)
        kb_p = np.zeros((MP, PADSH), np.int64)
        rows = r * SH + perm[r]
        ka_p[:, :SH] = kA[:, rows]
        kb_p[:, :SH] = kB[:, rows]
        KA = np.maximum(KA, ka_p.reshape(MP, NT, TILE).max(-1))
        KB = np.maximum(KB, kb_p.reshape(MP, NT, TILE).max(-1))
    KA = np.maximum(KA, 1)
    KB = np.maximum(KB, 1)

    # idx arrays per core / mp / window: [16, sum_t K*TILE/16] int16
    def wrap16(flat):
        return np.ascontiguousarray(flat.reshape(-1, 16).T)

    idx_arrays = [[[None, None] for _ in range(MP)] for _ in range(NC)]
    for r in range(NC):
        rows = r * SH + perm[r]
        for mp in range(MP):
            ts_r = np.zeros((PADSH, S), np.int64)
            ts_r[:SH] = ts[mp, rows]
            ka_r = np.zeros(PADSH, np.int64)
            ka_r[:SH] = kA[mp, rows]
            def kmajor(m):
                """[TILE, K] node-major -> k-major per 128-block:
                position (blk*K + k)*128 + j."""
                return np.concatenate(
                    [m[b * 128:(b + 1) * 128].T.ravel()
                     for b in range(TILE // 128)])

            partsA, partsB = [], []
            for t in range(NT):
                sl = slice(t * TILE, (t + 1) * TILE)
                ka_t = ka_r[sl][:, None]                     # [TILE,1]
                jA = np.arange(KA[mp][t])[None, :]
                a = np.where(jA < ka_t, ts_r[sl, :KA[mp][t]] if KA[mp][t] <= S
                             else np.pad(ts_r[sl], ((0, 0), (0, KA[mp][t] - S))),
                             c["ZA"])
                partsA.append(kmajor(a.astype(np.int64)))
                jB = np.arange(KB[mp][t])[None, :]
                src = np.take_along_axis(
                    ts_r[sl], np.minimum(ka_t + jB, S - 1), axis=-1)
                b = np.where(jB < (S - ka_t), src - WINB, c["ZB"])
                # dummy rows (ka=0 was set, S-ka=S -> would take garbage): mask
                if t * TILE + TILE > SH:
                    realn = max(0, SH - t * TILE)
                    b[realn:] = c["ZB"]
                partsB.append(kmajor(b.astype(np.int64)))
            fa = np.concatenate(partsA)
            fb = np.concatenate(partsB)
            assert fa.min() >= 0 and fa.max() < WINB
            assert fb.min() >= 0 and fb.max() <= 32767
            idx_arrays[r][mp][0] = wrap16(fa.astype(np.int16))
            idx_arrays[r][mp][1] = wrap16(fb.astype(np.int16))

    # weights (all bf16 on device)
    s = 1.0 / S
    bf = ml_dtypes.bfloat16
    wx1 = np.stack([np.concatenate([Wx1[mp, h] for h in range(c["H"])], -1)
                    for mp in range(MP)]).astype(bf)          # [MP,PREP,F]
    wn1 = (np.stack([np.concatenate([Wn1[mp, h] for h in range(c["H"])], -1)
                     for mp in range(MP)]) * s).astype(bf)
    rowperm = np.concatenate([
        np.arange(0, OUT),                       # xh0
        np.arange(2 * OUT, 3 * OUT),             # xh1
        np.arange(OUT, 2 * OUT),                 # nh0
        np.arange(3 * OUT, 4 * OUT),             # nh1
    ])
    wx2 = np.stack([np.concatenate([Wx2[mp, h] for h in range(c["H"])], -1)[rowperm]
                    for mp in range(MP)]).astype(bf)          # [MP,IN2,F]
    wn2 = (np.stack([np.concatenate([Wn2[mp, h] for h in range(c["H"])], -1)[rowperm]
                     for mp in range(MP)]) * s).astype(bf)

    secs, total16 = blob_layout(c, KA, KB)

    def put(blob, name, arr16):
        off, rows, cols = secs[name]
        assert arr16.shape == (rows, cols), (name, arr16.shape, (rows, cols))
        blob[off:off + rows * cols] = arr16.ravel()

    prepw16 = np.asarray(prep_W, np.float32).astype(bf).view(np.int16)
    wx1_16 = wx1.reshape(MP * c["PREP"], F).view(np.int16)
    wn1_16 = wn1.reshape(MP * c["PREP"], F).view(np.int16)
    wx2_16 = wx2.reshape(MP * c["IN2"], F).view(np.int16)
    wn2_16 = wn2.reshape(MP * c["IN2"], F).view(np.int16)

    feats_f = np.asarray(feats, np.float32)
    in_maps = []
    for r in range(NC):
        rows = r * SH + perm[r]
        fT = np.zeros((c["DIN"], PADSH), bf)
        fT[:, :SH] = feats_f[rows].astype(bf).T
        blob = np.zeros(total16, np.int16)
        put(blob, "featsT", fT.view(np.int16))
        put(blob, "prepW", prepw16)
        put(blob, "wx1", wx1_16)
        put(blob, "wn1", wn1_16)
        put(blob, "wx2", wx2_16)
        put(blob, "wn2", wn2_16)
        for mp in range(MP):
            put(blob, f"idxa{mp}", idx_arrays[r][mp][0])
            put(blob, f"idxb{mp}", idx_arrays[r][mp][1])
        in_maps.append({"blob": blob})
    return c, KA, KB, perm, in_maps


# ---------------------------------------------------------------- device program

def build_program(c, KA, KB, num_queues=4, do_ag=True, do_gather=True,
                  do_mm=True, do_wout=True):
    MP, NT, TILE, PADSH = c["MP"], c["NT"], c["TILE"], c["PADSH"]
    F, PREP, DIN, IN2 = c["F"], c["PREP"], c["DIN"], c["IN2"]
    CON, NTOK, STRIPES, WINB, SH = (c["CON"], c["NTOK"], c["STRIPES"],
                                    c["WINB"], c["SH"])
    OUTW = 2 * IN2                                     # 512 output cols

    secs, total16 = blob_layout(c, KA, KB)

    nc = bacc.Bacc("TRN2", debug=False, num_swdge_queues=num_queues)

    blob = nc.dram_tensor("blob", [total16], I16, kind="ExternalInput")
    out = nc.dram_tensor("out", [MP, PADSH, OUTW], BF16, kind="ExternalOutput")

    def sec(name, dt=BF16):
        off, rows, cols = secs[name]
        ap = blob[off:off + rows * cols].rearrange("(r c) -> r c", c=cols)
        return ap.bitcast(dt)

    # per-mp tables in DRAM; gathers read them directly (no SBUF table)
    y1in = [nc.dram_tensor(f"y1in{mp}", [CON, F], BF16) for mp in range(MP)]
    t1out = [nc.dram_tensor(f"t1out{mp}", [NTOK, F], BF16, addr_space="Shared")
             for mp in range(MP)]
    y2in = [nc.dram_tensor(f"y2in{mp}", [CON, F], BF16) for mp in range(MP)]
    t2out = [nc.dram_tensor(f"t2out{mp}", [NTOK, F], BF16, addr_space="Shared")
             for mp in range(MP)]

    KAmax = int(max(KA.max(), KB.max()))
    rg = [list(range(c["NC"]))]

    nc.gpsimd.load_library(library_config.mlp)
    with tile.TileContext(nc) as tc:
        with (
            tc.tile_pool(name="const", bufs=1) as cpool,
            tc.tile_pool(name="big", bufs=1) as big,
            tc.tile_pool(name="f", bufs=3) as fpool,
            tc.tile_pool(name="g", bufs=4) as gpool,
            tc.tile_pool(name="i", bufs=2) as ipool,
            tc.tile_pool(name="a", bufs=6) as apool,
            tc.tile_pool(name="st", bufs=6) as spool,
            tc.tile_pool(name="pmm", bufs=4, space="PSUM") as pmm,
            tc.tile_pool(name="ptr", bufs=1, space="PSUM") as ptr,
            tc.tile_pool(name="ptr2", bufs=1, space="PSUM") as ptr2,
        ):
            identb = cpool.tile([128, 128], BF16, tag="identb")
            make_identity(nc, identb[:])
            prepw_t = [cpool.tile([128, PREP], BF16, tag=f"prepw{k}",
                                  name=f"prepw{k}") for k in range(DIN // 128)]
            prepW_ap = sec("prepW")
            for k in range(DIN // 128):
                nc.sync.dma_start(out=prepw_t[k][:],
                                  in_=prepW_ap[k * 128:(k + 1) * 128, :])
            wx1_t = [cpool.tile([128, F], BF16, tag=f"wx1{mp}", name=f"wx1t{mp}")
                     for mp in range(MP)]
            wn1_t = [cpool.tile([128, F], BF16, tag=f"wn1{mp}", name=f"wn1t{mp}")
                     for mp in range(MP)]
            wx2_t = [[cpool.tile([128, F], BF16, tag=f"wx2{mp}{k}", name=f"wx2t{mp}{k}")
                      for k in range(2)] for mp in range(MP)]
            wn2_t = [[cpool.tile([128, F], BF16, tag=f"wn2{mp}{k}", name=f"wn2t{mp}{k}")
                      for k in range(2)] for mp in range(MP)]
            wx1_ap, wn1_ap = sec("wx1"), sec("wn1")
            wx2_ap, wn2_ap = sec("wx2"), sec("wn2")
            for mp in range(MP):
                nc.sync.dma_start(out=wx1_t[mp][:],
                                  in_=wx1_ap[mp * PREP:(mp + 1) * PREP, :])
                nc.sync.dma_start(out=wn1_t[mp][:],
                                  in_=wn1_ap[mp * PREP:(mp + 1) * PREP, :])
                for k in range(2):
                    r0 = mp * IN2 + k * 128
                    nc.sync.dma_start(out=wx2_t[mp][k][:],
                                      in_=wx2_ap[r0:r0 + 128, :])
                    nc.sync.dma_start(out=wn2_t[mp][k][:],
                                      in_=wn2_ap[r0:r0 + 128, :])

            zrow = cpool.tile([128, F], BF16, tag="zrow")
            nc.any.memset(zrow[:], 0.0)
            zpad = CON - SH
            for mp in range(MP):
                nc.sync.dma_start(out=y1in[mp][SH:CON, :], in_=zrow[:zpad, :])
                nc.sync.dma_start(out=y2in[mp][SH:CON, :], in_=zrow[:zpad, :])

            h0T = big.tile([128, PADSH], BF16, tag="h0T")
            h1x = [big.tile([128, PADSH], BF16, tag=f"h1x{mp}", name=f"h1x{mp}")
                   for mp in range(MP)]
            h1n = [big.tile([128, PADSH], BF16, tag=f"h1n{mp}", name=f"h1n{mp}")
                   for mp in range(MP)]

            featsT_ap = sec("featsT")
            idx_aps = {(mp, 0): sec(f"idxa{mp}", I16) for mp in range(MP)}
            idx_aps.update({(mp, 1): sec(f"idxb{mp}", I16) for mp in range(MP)})

            def write_y(ysb, yin, t):
                """ysb: [128 f, TILE n] bf16 -> transpose chunks -> yin rows."""
                for half in range(TILE // 128):
                    r0 = t * TILE + half * 128
                    nrows = max(0, min(128, SH - r0))
                    if nrows == 0:
                        continue
                    trp = ptr.tile([128, 128], BF16, tag="trb")
                    nc.tensor.transpose(trp[:], ysb[:, half * 128:half * 128 + 128],
                                        identb[:])
                    stg = spool.tile([128, 128], BF16, tag="yst")
                    nc.any.tensor_copy(out=stg[:], in_=trp[:])
                    nc.sync.dma_start(out=yin[r0:r0 + nrows, :], in_=stg[:nrows, :])

            # ---------------- phase P: h0T, y1 contributions
            # mp0 contributions first so AG(y1-0) can start mid-P
            def y1_pass(mp):
                for t in range(NT):
                    sl = slice(t * TILE, (t + 1) * TILE)
                    yps = pmm.tile([128, TILE], F32, tag="mm")
                    nc.tensor.matmul(out=yps[:], lhsT=wn1_t[mp][:],
                                     rhs=h0T[:, sl], start=True, stop=True)
                    ysb = spool.tile([128, TILE], BF16, tag="ybf")
                    nc.any.tensor_copy(out=ysb[:], in_=yps[:])
                    write_y(ysb, y1in[mp], t)

            for t in range(NT):
                sl = slice(t * TILE, (t + 1) * TILE)
                f0 = fpool.tile([128, TILE], BF16, tag="f0")
                f1 = fpool.tile([128, TILE], BF16, tag="f1")
                nc.sync.dma_start(out=f0[:], in_=featsT_ap[0:128, sl])
                nc.sync.dma_start(out=f1[:], in_=featsT_ap[128:256, sl])
                h0ps = pmm.tile([128, TILE], F32, tag="mm")
                nc.tensor.matmul(out=h0ps[:], lhsT=prepw_t[0][:], rhs=f0[:],
                                 start=True, stop=False)
                nc.tensor.matmul(out=h0ps[:], lhsT=prepw_t[1][:], rhs=f1[:],
                                 start=False, stop=True)
                nc.any.tensor_copy(out=h0T[:, sl], in_=h0ps[:])
            y1_pass(0)
            if do_ag:
                nc.gpsimd.collective_compute(
                    "AllGather", mybir.AluOpType.bypass, replica_groups=rg,
                    ins=[y1in[0][:]], outs=[t1out[0][:]])
            y1_pass(1)
            if do_ag:
                nc.gpsimd.collective_compute(
                    "AllGather", mybir.AluOpType.bypass, replica_groups=rg,
                    ins=[y1in[1][:]], outs=[t1out[1][:]])

            NB = TILE // 128                   # 128-node blocks per tile
            ILEN = {(mp, w): int((KA if w == 0 else KB)[mp].sum()) * TILE // 16
                    for mp in range(MP) for w in range(2)}
            IMAX = max(ILEN.values())

            def load_phase_idx(mp):
                """One broadcast DMA per window for the whole phase's indices."""
                tiles = {}
                for w in range(2):
                    it = ipool.tile([128, IMAX], I16, tag=f"idx{w}",
                                    name=f"idx{w}")
                    nc.sync.dma_start(
                        out=it[:, :ILEN[(mp, w)]],
                        in_=idx_aps[(mp, w)][:, :ILEN[(mp, w)]]
                        .unsqueeze(0).broadcast_to([8, 16, ILEN[(mp, w)]]))
                    tiles[w] = it
                return tiles

            def gather_pair(tsrc, mp, win, t, off, itiles):
                """Non-transpose gather from the DRAM table; k-major positions
                (blk*K + k)*128 + j -> node j of block blk gets its K neighbor
                rows contiguously on partition j. Strided reduce over k."""
                K = int((KA if win == 0 else KB)[mp][t])
                nidx = K * TILE
                it = itiles[win]
                agg = apool.tile([128, NB * F], F32, tag="agg")
                if do_gather:
                    g = gpool.tile([128, KAmax * NB * F], BF16, tag="g")
                    src = tsrc[:] if win == 0 else tsrc[WINB:, :]
                    nc.gpsimd.dma_gather(
                        out_ap=g[:, :K * NB * F].rearrange(
                            "p (r f) -> p r f", f=F),
                        in_ap=src,
                        idxs_ap=it[:, off:off + nidx // 16],
                        num_idxs=nidx,
                        num_idxs_reg=nidx,
                        elem_size=F,
                        transpose=False,
                        single_packet=False,
                        queue_num=3,
                    )
                    nc.vector.tensor_reduce(
                        out=agg[:].rearrange("p (b f) -> p b f", b=NB),
                        in_=g[:, :K * NB * F].rearrange(
                            "p (b k f) -> p b f k", b=NB, k=K),
                        axis=mybir.AxisListType.X, op=mybir.AluOpType.add)
                else:
                    nc.any.memset(agg[:], 0.0)
                return agg

            def write_out(xh_f, nh_nm, mp, t, half, lay):
                """xh_f [128 f, TILE n] feature-major; nh_nm [128 n, NB*128]
                node-major (block-major cols). Assemble node-major 256-col rows
                ([xh0|nh0|xh1|nh1] 64-col pieces) and write one contiguous 512B
                chunk per node into out[mp, rows, lay*256:(lay+1)*256]."""
                fsl = slice(half * 128, half * 128 + 128)
                pp = ptr if half == 0 else ptr2
                trx = pp.tile([128, 128], BF16, tag="trx", name="trx")
                nc.tensor.transpose(trx[:], xh_f[:, fsl], identb[:])
                stg = spool.tile([128, 256], BF16, tag="ost")
                stg4 = stg[:].rearrange("n (h w c) -> n h w c", h=2, w=2)
                nc.any.tensor_copy(
                    out=stg4[:, :, 0, :],
                    in_=trx[:].rearrange("n (h c) -> n h c", h=2))
                nc.any.tensor_copy(
                    out=stg4[:, :, 1, :],
                    in_=nh_nm[:, fsl].rearrange("n (h c) -> n h c", h=2))
                r0 = t * TILE + half * 128
                if do_wout:
                    nc.sync.dma_start(
                        out=out[mp, r0:r0 + 128, lay * 256:(lay + 1) * 256],
                        in_=stg[:])

            def agg_tile(tsrc, mp, t, offa, offb, itiles):
                """Gather+reduce both windows -> relu'd node-major nh [128, TILE]."""
                aggA = gather_pair(tsrc, mp, 0, t, offa, itiles)
                aggB = gather_pair(tsrc, mp, 1, t, offb, itiles)
                aggS = apool.tile([128, NB * F], F32, tag="aggs")
                nc.vector.tensor_add(out=aggS[:], in0=aggA[:], in1=aggB[:])
                nh_nm = apool.tile([128, NB * F], BF16, tag="nhnm")
                nc.scalar.activation(out=nh_nm[:], in_=aggS[:],
                                     func=mybir.ActivationFunctionType.Relu)
                return nh_nm

            def gather_phase_l1(mp):
                itiles = load_phase_idx(mp)
                offa = offb = 0
                for t in range(NT):
                    sl = slice(t * TILE, (t + 1) * TILE)
                    nh_nm = agg_tile(t1out[mp], mp, t, offa, offb, itiles)
                    offa += int(KA[mp][t]) * TILE // 16
                    offb += int(KB[mp][t]) * TILE // 16
                    xh_f = apool.tile([128, TILE], BF16, tag="xhf")
                    if do_mm:
                        xhps = pmm.tile([128, TILE], F32, tag="mm")
                        nc.tensor.matmul(out=xhps[:], lhsT=wx1_t[mp][:],
                                         rhs=h0T[:, sl], start=True, stop=True)
                        nc.scalar.activation(out=xh_f[:], in_=xhps[:],
                                             func=mybir.ActivationFunctionType.Relu)
                    else:
                        nc.any.tensor_copy(out=xh_f[:], in_=nh_nm[:])
                    nc.any.tensor_copy(out=h1x[mp][:, sl], in_=xh_f[:])
                    # h1n needs feature-major: transpose each node-major block
                    for half in range(NB):
                        trn = ptr.tile([128, 128], BF16, tag="trn", name="trn")
                        nc.tensor.transpose(
                            trn[:], nh_nm[:, half * 128:half * 128 + 128],
                            identb[:])
                        nc.any.tensor_copy(
                            out=h1n[mp][:, t * TILE + half * 128:
                                        t * TILE + half * 128 + 128],
                            in_=trn[:])
                        write_out(xh_f, nh_nm, mp, t, half, 0)
                    y2ps = pmm.tile([128, TILE], F32, tag="mm")
                    nc.tensor.matmul(out=y2ps[:], lhsT=wn2_t[mp][0][:],
                                     rhs=h1x[mp][:, sl], start=True, stop=False)
                    nc.tensor.matmul(out=y2ps[:], lhsT=wn2_t[mp][1][:],
                                     rhs=h1n[mp][:, sl], start=False, stop=True)
                    ysb = spool.tile([128, TILE], BF16, tag="ybf")
                    nc.any.tensor_copy(out=ysb[:], in_=y2ps[:])
                    write_y(ysb, y2in[mp], t)

            def gather_phase_l2(mp, xh_pre=False):
                itiles = load_phase_idx(mp)
                offa = offb = 0
                for t in range(NT):
                    sl = slice(t * TILE, (t + 1) * TILE)
                    nh_nm = agg_tile(t2out[mp], mp, t, offa, offb, itiles)
                    offa += int(KA[mp][t]) * TILE // 16
                    offb += int(KB[mp][t]) * TILE // 16
                    if xh_pre:
                        xh_f = h0T[:, sl]      # precomputed relu'd xh2 (see below)
                    elif do_mm:
                        xhps = pmm.tile([128, TILE], F32, tag="mm")
                        nc.tensor.matmul(out=xhps[:], lhsT=wx2_t[mp][0][:],
                                         rhs=h1x[mp][:, sl], start=True, stop=False)
                        nc.tensor.matmul(out=xhps[:], lhsT=wx2_t[mp][1][:],
                                         rhs=h1n[mp][:, sl], start=False, stop=True)
                        xh_t = apool.tile([128, TILE], BF16, tag="xhf")
                        nc.scalar.activation(out=xh_t[:], in_=xhps[:],
                                             func=mybir.ActivationFunctionType.Relu)
                        xh_f = xh_t[:]
                    else:
                        xh_f = nh_nm
                    for half in range(NB):
                        write_out(xh_f, nh_nm, mp, t, half, 1)

            # schedule: P -> AG(y1-0), AG(y1-1) -> L1(0) -> AG(y2-0) -> L1(1)
            # -> AG(y2-1) -> [xh2-mp0 into h0T] -> L2(0) -> L2(1)
            gather_phase_l1(0)
            if do_ag:
                nc.gpsimd.collective_compute(
                    "AllGather", mybir.AluOpType.bypass, replica_groups=rg,
                    ins=[y2in[0][:]], outs=[t2out[0][:]])
            gather_phase_l1(1)
            if do_ag:
                nc.gpsimd.collective_compute(
                    "AllGather", mybir.AluOpType.bypass, replica_groups=rg,
                    ins=[y2in[1][:]], outs=[t2out[1][:]])
            if do_mm:
                # xh2 for mp0 (independent of AG(y2-1)): h0T is dead after L1
                for t in range(NT):
                    sl = slice(t * TILE, (t + 1) * TILE)
                    xhps = pmm.tile([128, TILE], F32, tag="mm")
                    nc.tensor.matmul(out=xhps[:], lhsT=wx2_t[0][0][:],
                                     rhs=h1x[0][:, sl], start=True, stop=False)
                    nc.tensor.matmul(out=xhps[:], lhsT=wx2_t[0][1][:],
                                     rhs=h1n[0][:, sl], start=False, stop=True)
                    nc.scalar.activation(out=h0T[:, sl], in_=xhps[:],
                                         func=mybir.ActivationFunctionType.Relu)
            gather_phase_l2(0, xh_pre=do_mm)
            gather_phase_l2(1)
    nc.compile()
    return nc


def timed_run(nc, in_maps, n_cores, iters=(1, 9)):
    """Estimate device exec time via slope: dispatch K back-to-back executions
    with device-resident inputs and donation-chained outputs; block once.

    Returns (results_list, est_ns).
    """
    import jax
    import numpy as np
    from jax.sharding import Mesh, PartitionSpec
    from jax.experimental.shard_map import shard_map
    from concourse import bass2jax
    from concourse.bass2jax import _bass_exec_p, partition_id_tensor
    import time as _time

    bass2jax.install_neuronx_cc_hook()
    partition_name = nc.partition_id_tensor.name if nc.partition_id_tensor else None
    in_names, out_names, out_avals = [], [], []
    import concourse.mybir as mybir_
    for alloc in nc.m.functions[0].allocations:
        if not isinstance(alloc, mybir_.MemoryLocationSet):
            continue
        name = alloc.memorylocations[0].name
        if alloc.kind == "ExternalInput":
            if name != partition_name:
                in_names.append(name)
        elif alloc.kind == "ExternalOutput":
            out_names.append(name)
            out_avals.append(jax.core.ShapedArray(
                tuple(alloc.tensor_shape), mybir_.dt.np(alloc.dtype)))
    n_params = len(in_names)
    all_in_names = list(in_names) + list(out_names)
    if partition_name is not None:
        all_in_names.append(partition_name)

    def _body(*args):
        operands = list(args)
        if partition_name is not None:
            operands.append(partition_id_tensor())
        return tuple(_bass_exec_p.bind(
            *operands,
            out_avals=tuple(out_avals),
            in_names=tuple(all_in_names),
            out_names=tuple(out_names),
            lowering_input_output_aliases=(),
            sim_require_finite=True, sim_require_nnan=True, nc=nc))

    n_outs = len(out_names)
    donate = tuple(range(n_params, n_params + n_outs))
    devices = jax.devices()[:n_cores]
    mesh = Mesh(np.asarray(devices), ("core",))
    sharded = jax.jit(
        shard_map(_body, mesh=mesh,
                  in_specs=(PartitionSpec("core"),) * (n_params + n_outs),
                  out_specs=(PartitionSpec("core"),) * n_outs, check_rep=False),
        donate_argnums=donate, keep_unused=True)

    concat_in = [np.concatenate([np.asarray(m[name]) for m in in_maps], axis=0)
                 for name in in_names]
    dev_in = [jax.device_put(a) for a in concat_in]
    zeros = [jax.device_put(np.zeros((n_cores * a.shape[0], *a.shape[1:]),
                                     a.dtype)) for a in out_avals]
    outs = sharded(*dev_in, *zeros)
    jax.block_until_ready(outs)
    results_arr = [np.asarray(o) for o in outs]

    def run_k(k):
        nonlocal outs
        t0 = _time.perf_counter()
        for _ in range(k):
            outs = sharded(*dev_in, *outs)
        jax.block_until_ready(outs)
        return _time.perf_counter() - t0

    k0, k1 = iters
    run_k(1)
    t_lo = min(run_k(k0) for _ in range(3))
    t_hi = min(run_k(k1) for _ in range(3))
    est = (t_hi - t_lo) / (k1 - k0)
    results = [
        {name: results_arr[i].reshape(n_cores, *out_avals[i].shape)[c]
         for i, name in enumerate(out_names)}
        for c in range(n_cores)]
    return results, est * 1e9


# ---------------------------------------------------------------- entry

def run(cfg, feats, prep_W, Wx1, Wn1, Wx2, Wn2, node_neigh, num_queues=4,
        nc_cache=None):
    c, KA, KB, perm, in_maps = host_prep(
        cfg, feats, prep_W, Wx1, Wn1, Wx2, Wn2, node_neigh)
    key = (KA.tobytes(), KB.tobytes(), num_queues)
    if nc_cache is not None and nc_cache.get("key") == key:
        nc = nc_cache["nc"]
    else:
        nc = build_program(c, KA, KB, num_queues=num_queues)
        if nc_cache is not None:
            nc_cache["key"] = key
            nc_cache["nc"] = nc
    res = run_bass_kernel_spmd(nc, in_maps, list(range(c["NC"])))
    MP, SH, N = c["MP"], c["SH"], c["N"]
    outw = 2 * c["IN2"]
    full = np.empty((MP, N, outw), np.float32)
    for r in range(c["NC"]):
        o = res.results[r]["out"]                     # [MP, PADSH, 512] bf16
        full[:, r * SH + perm[r], :] = o[:, :SH, :].astype(np.float32)
    return full, res


# ---------------------------------------------------------------- harness entry

_NC_CACHE = {}


def kernel(**inputs):
    """Full-input GNN kernel: shards across 8 NeuronCores internally.

    inputs: feats [50000,256] f32, prep_W [256,128] f32,
            Wx1/Wn1 [2,2,128,64] f32, Wx2/Wn2 [2,2,256,64] f32,
            node_neigh [2,50000,16] int32
    returns [2, 50000, 512] float32
    """
    cfg = default_cfg()
    full, _ = run(cfg, inputs["feats"], inputs["prep_W"], inputs["Wx1"],
                  inputs["Wn1"], inputs["Wx2"], inputs["Wn2"],
                  inputs["node_neigh"], num_queues=4, nc_cache=_NC_CACHE)
    return full
